# revision 9
# baseline (speedup 1.0000x reference)
"""DiSA (directional self-attention) Bass kernel for Trainium2, 8 cores.

Math (per batch b):
  rep = elu(inputs @ W_fc.T + b_fc)                       [S, D]
  dep = rep @ W1.T ; head = rep @ W2.T                    [S, D]
  logits[i,j,d] = C*tanh((dep[j,d] + head[i,d] + b1[d])/C)
  mask[i,j] = rep_mask[j] * (j > i)
  attn = masked softmax over j, per (i, d) channel  (shift-invariance:
         logits bounded in [-C, C], so no max-subtract needed)
  attn_res[i,d] = sum_j attn * rep[j,d]
  gate = sigmoid(rep @ W_f1.T + attn_res @ W_f2.T + b_f)
       = 0.5 + 0.5*tanh(0.5*z)
  out = (gate*rep + (1-gate)*attn_res) * rep_mask[i]
      = 0.5*rep_mask[i] * ((rep+attn_res) + tanh(0.5*z)*(rep-attn_res))

Sharding: core c -> batch b=c//2, d-half h=c%2 (planes d in [150h, 150h+150)).
Each core computes the full out[b].T (phase C duplicated in the pair after a
pairwise AllGather of attn_res.T); the host takes core 2b's output.

Per-d-plane layout: [j (partitions), i (free)].  exp(masked logits) is built
as exp(C*tanh(x/C) + logmask[j]) (rep_mask folded into the ACT bias); the
strict-upper triangle mask (j > i) is a constant fp16 multiply; both
softmax reductions over j (sum of e, sum of e*rep) are PE matmuls with the
masked-exp tile as the stationary operand and [ones | rep_col] as a 2-column
moving operand, so results land as [i, 2] PSUM columns.
"""

import numpy as np

B, S, D = 4, 256, 300
C = 5.0
HALF = D // 2          # 150 d-planes per core
G = 6                  # planes per group
NG = HALF // G         # 25 groups
NEG = -30000.0         # exp(x + NEG) == 0 in fp32

_CACHE: dict = {}


def _chunks(total, step=128):
    return [(s, min(step, total - s)) for s in range(0, total, step)]


def _build_nc():
    import concourse.bass as bass
    import concourse.tile as tile
    from concourse import bacc, mybir

    F32 = mybir.dt.float32
    F32R = mybir.dt.float32r
    F16 = mybir.dt.float16
    AF = mybir.ActivationFunctionType
    OP = mybir.AluOpType

    nc = bacc.Bacc("TRN2", target_bir_lowering=False, debug=False, num_devices=8)

    def din(name, shape, dt=F32):
        return nc.dram_tensor(name, shape, dt, kind="ExternalInput").ap()

    inputsT_d = din("inputsT", [D, S], F32R)    # inputs[b].T
    W_fcT_d = din("W_fcT", [D, D], F32R)        # [e, h]
    W_fcTh_d = din("W_fcTh", [D, HALF], F32R)   # W_fc.T[:, half]
    b_fch_d = din("b_fch_row", [1, HALF], F32R)
    ones_d = din("ones_row", [1, D], F32R)
    ident_d = din("ident", [128, 128])
    W1T_d = din("W1Th", [D, HALF], F32R)        # W1.T[:, half]
    W2T_d = din("W2Th", [D, HALF], F32R)
    W_f1T_d = din("W_f1T", [D, D], F32R)
    W_f2T_d = din("W_f2T", [D, D], F32R)
    b_fc_d = din("b_fc_row", [1, D], F32R)
    b1h_d = din("b1h_row", [1, HALF], F32R)
    b_f_d = din("b_f_row", [1, D], F32R)
    logm_d = din("logm", [S, 1])                # (rep_mask-1)*30000
    mh_d = din("mh_row", [1, S])                # 0.5*rep_mask
    tri0_d = din("tri0", [128, G * 128], F16)   # (j>i) tile, j in [0,128)
    tri1_d = din("tri1", [128, G * S], F16)     # (j>i), j in [128,256)
    outT_d = nc.dram_tensor("outT", [D, S], F32, kind="ExternalOutput").ap()

    DC = _chunks(D)          # [(0,128),(128,128),(256,44)]
    DM = _chunks(HALF)       # [(0,128),(128,22)]

    with tile.TileContext(nc) as tc:
        # ---------- persistent tiles ----------
        with (
            tc.tile_pool(name="persist", bufs=1) as pp,
            tc.tile_pool(name="sumsw", bufs=1) as swp,
        ):
            ones_row = pp.tile([1, D], F32R)
            nc.sync.dma_start(ones_row[:], ones_d[:])

            inT = [pp.tile([n, S], F32R, tag=f"inT{i}", name=f"inT{i}") for i, (o, n) in enumerate(DC)]
            WfcT = [pp.tile([n, D], F32R, tag=f"wfc{i}", name=f"wfc{i}") for i, (o, n) in enumerate(DC)]
            WfcTh = [pp.tile([n, HALF], F32R, tag=f"wfch{i}", name=f"wfch{i}") for i, (o, n) in enumerate(DC)]
            W1T = [pp.tile([n, HALF], F32R, tag=f"w1{i}", name=f"w1_{i}") for i, (o, n) in enumerate(DC)]
            W2T = [pp.tile([n, HALF], F32R, tag=f"w2{i}", name=f"w2_{i}") for i, (o, n) in enumerate(DC)]
            Wf1T = [pp.tile([n, D], F32R, tag=f"wg1{i}", name=f"wg1_{i}") for i, (o, n) in enumerate(DC)]
            Wf2T = [pp.tile([n, D], F32R, tag=f"wg2{i}", name=f"wg2_{i}") for i, (o, n) in enumerate(DC)]
            for i, (o, n) in enumerate(DC):
                nc.sync.dma_start(inT[i][:], inputsT_d[o : o + n, :])
                nc.sync.dma_start(WfcT[i][:], W_fcT_d[o : o + n, :])
                nc.sync.dma_start(WfcTh[i][:], W_fcTh_d[o : o + n, :])
                nc.sync.dma_start(W1T[i][:], W1T_d[o : o + n, :])
                nc.sync.dma_start(W2T[i][:], W2T_d[o : o + n, :])
                nc.sync.dma_start(Wf1T[i][:], W_f1T_d[o : o + n, :])
                nc.sync.dma_start(Wf2T[i][:], W_f2T_d[o : o + n, :])
            b_fc_row = pp.tile([1, D], F32R)
            nc.sync.dma_start(b_fc_row[:], b_fc_d[:])
            b_fch_row = pp.tile([1, HALF], F32R)
            nc.sync.dma_start(b_fch_row[:], b_fch_d[:])
            ident = pp.tile([128, 128], F32)
            nc.sync.dma_start(ident[:], ident_d[:])
            b1h_row = pp.tile([1, HALF], F32R)
            nc.sync.dma_start(b1h_row[:], b1h_d[:])
            b_f_row = pp.tile([1, D], F32R)
            nc.sync.dma_start(b_f_row[:], b_f_d[:])
            logm0 = pp.tile([128, 1], F32)
            nc.sync.dma_start(logm0[:], logm_d[0:128, :])
            logm1 = pp.tile([128, 1], F32)
            nc.sync.dma_start(logm1[:], logm_d[128:256, :])
            mh_row = pp.tile([1, S], F32)
            nc.sync.dma_start(mh_row[:], mh_d[:])
            tri0 = pp.tile([128, G * 128], F16)
            nc.sync.dma_start(tri0[:], tri0_d[:])
            tri1 = pp.tile([128, G * S], F16)
            nc.sync.dma_start(tri1[:], tri1_d[:])

            # phase A outputs (persist through B/C)
            repT = [pp.tile([n, S], F32R, tag=f"repT{i}", name=f"repT{i}") for i, (o, n) in enumerate(DC)]
            rep_nat = [pp.tile([128, HALF], F32, tag=f"repn{i}", name=f"repn{i}") for i in range(2)]
            depT = [pp.tile([n, S], F32R, tag=f"depT{i}", name=f"depT{i}") for i, (o, n) in enumerate(DM)]
            headT = [pp.tile([n, S], F32R, tag=f"headT{i}", name=f"headT{i}") for i, (o, n) in enumerate(DM)]
            dep_c0 = pp.tile([128, HALF], F32)     # dep natural, j in [0,128)
            il = [pp.tile([128, 2 * HALF], F16, tag=f"il{i}", name=f"il{i}") for i in range(2)]
            # phase B accumulators: cols (d_local, {sums, W})
            sumsW = [swp.tile([128, 2 * HALF], F32, tag=f"sw{i}", name=f"sw{i}") for i in range(2)]

            # ---------- phase A ----------
            with (
                tc.tile_pool(name="pa_ps", bufs=2, space="PSUM") as pa_ps,
                tc.tile_pool(name="pa_sb", bufs=2) as pa_sb,
            ):
                def elu_from_psum(ps_ap, out_ap, n):
                    # out = relu(x) + exp(min(x, 0)) - 1   (b_fc added in PSUM)
                    relu_t = pa_sb.tile([n, ps_ap.shape[1]], F32, tag="elu_r", name="elu_r")
                    nc.scalar.activation(relu_t[:], ps_ap, AF.Relu)
                    min_t = pa_sb.tile([n, ps_ap.shape[1]], F32, tag="elu_m", name="elu_m")
                    nc.vector.tensor_scalar(
                        out=min_t[:], in0=ps_ap, scalar1=0.0, scalar2=None, op0=OP.min
                    )
                    exp_t = pa_sb.tile([n, ps_ap.shape[1]], F32, tag="elu_e", name="elu_e")
                    nc.scalar.activation(exp_t[:], min_t[:], AF.Exp)
                    nc.vector.scalar_tensor_tensor(
                        out=out_ap, in0=exp_t[:], scalar=-1.0, in1=relu_t[:],
                        op0=OP.add, op1=OP.add,
                    )

                # rep^T [d, s] = elu(W_fcT.T @ inputsT + b_fc)
                for i, (o, n) in enumerate(DC):
                    ps = pa_ps.tile([n, S], F32, tag="paT", name="paT")
                    for k, (eo, en) in enumerate(DC):
                        nc.tensor.matmul(
                            ps[:], WfcT[k][:, o : o + n],
                            inT[k][:],
                            start=(k == 0), stop=False,
                        )
                    nc.tensor.matmul(
                        ps[:], b_fc_row[0:1, o : o + n],
                        ones_row[0:1, 0:S],
                        start=False, stop=True,
                    )
                    elu_from_psum(ps[:], repT[i][:], n)

                # rep natural half [s-chunk, d_local] = elu(inputsT.T @ W_fcTh + b_fch)
                for i in range(2):
                    so = 128 * i
                    ps = pa_ps.tile([128, HALF], F32, tag="paN", name="paN")
                    for k, (eo, en) in enumerate(DC):
                        nc.tensor.matmul(
                            ps[:], inT[k][:, so : so + 128],
                            WfcTh[k][:],
                            start=(k == 0), stop=False,
                        )
                    nc.tensor.matmul(
                        ps[:], ones_row[0:1, 0:128],
                        b_fch_row[:],
                        start=False, stop=True,
                    )
                    elu_from_psum(ps[:], rep_nat[i][:], 128)

                # interleave [ones | rep] fp16, per j-chunk
                for i in range(2):
                    v3 = il[i][:].rearrange("p (d two) -> p d two", two=2)
                    nc.vector.memset(v3[:, :, 0:1], 1.0)
                    nc.vector.tensor_copy(
                        v3[:, :, 1:2],
                        rep_nat[i][:].unsqueeze(2),
                    )

                # dep^T / head^T [d_local, s]
                for i, (o, n) in enumerate(DM):
                    ps = pa_ps.tile([n, S], F32, tag="paT", name="paT")
                    for k, (ho, hn) in enumerate(DC):
                        nc.tensor.matmul(
                            ps[:], W1T[k][:, o : o + n],
                            repT[k][:],
                            start=(k == 0), stop=(k == 2),
                        )
                    nc.vector.tensor_copy(depT[i][:], ps[:])

                    ps2 = pa_ps.tile([n, S], F32, tag="paT", name="paT")
                    for k, (ho, hn) in enumerate(DC):
                        nc.tensor.matmul(
                            ps2[:], W2T[k][:, o : o + n],
                            repT[k][:],
                            start=(k == 0), stop=False,
                        )
                    nc.tensor.matmul(
                        ps2[:], b1h_row[0:1, o : o + n],
                        ones_row[0:1, 0:S],
                        start=False, stop=True,
                    )
                    nc.vector.tensor_copy(headT[i][:], ps2[:])

                # dep natural c0 [j in 0:128, d_local]
                ps = pa_ps.tile([128, HALF], F32, tag="paN", name="paN")
                for k, (ho, hn) in enumerate(DC):
                    nc.tensor.matmul(
                        ps[:], repT[k][:, 0:128], W1T[k][:],
                        start=(k == 0), stop=(k == 2),
                    )
                nc.vector.tensor_copy(dep_c0[:], ps[:])

            # ---------- phase B ----------
            def rows_of(tiles, lo, hi):
                """Split [lo,hi) d_local rows across the DM tiles."""
                segs = []
                for i, (o, n) in enumerate(DM):
                    a, b2 = max(lo, o), min(hi, o + n)
                    if a < b2:
                        segs.append((tiles[i], a - o, b2 - a))
                return segs

            with (
                tc.tile_pool(name="stA", bufs=2) as stA_p,
                tc.tile_pool(name="stB", bufs=2) as stB_p,
                tc.tile_pool(name="Hb", bufs=3) as H_p,
                tc.tile_pool(name="xc0", bufs=2) as xc0_p,
                tc.tile_pool(name="xps", bufs=2, space="PSUM") as xps_p,
                tc.tile_pool(name="redps", bufs=2, space="PSUM") as red_p,
                tc.tile_pool(name="tc0", bufs=2) as tc0_p,
                tc.tile_pool(name="tc1", bufs=2) as tc1_p,
                tc.tile_pool(name="ec0", bufs=2) as ec0_p,
                tc.tile_pool(name="ec1", bufs=2) as ec1_p,
            ):
                for grp in range(NG):
                    d0 = grp * G
                    stageA = stA_p.tile([1, G * S], F32R)
                    off = 0
                    for t, ro, rn in rows_of(headT, d0, d0 + G):
                        nc.sync.dma_start(
                            stageA[0:1, off : off + rn * S], t[ro : ro + rn, :]
                        )
                        off += rn * S
                    stageB = stB_p.tile([1, G * 128], F32R)
                    off = 0
                    for t, ro, rn in rows_of(depT, d0, d0 + G):
                        nc.sync.dma_start(
                            stageB[0:1, off : off + rn * 128], t[ro : ro + rn, 128:S]
                        )
                        off += rn * 128

                    x_ps = xps_p.tile([128, G * S], F32)
                    xc0 = xc0_p.tile([128, G * 128], F32)
                    for p in range(G):
                        o1 = p * S
                        nc.tensor.matmul(
                            x_ps[:, o1 : o1 + S],
                            ones_row[0:1, 0:128],
                            stageA[0:1, o1 : o1 + S],
                            start=True, stop=False,
                        )
                        nc.tensor.matmul(
                            x_ps[:, o1 : o1 + S],
                            stageB[0:1, p * 128 : (p + 1) * 128],
                            ones_row[0:1, 0:S],
                            start=False, stop=True,
                        )
                        H = H_p.tile([128, 128], F32R)
                        nc.gpsimd.partition_broadcast(
                            H[:], stageA[0:1, o1 : o1 + 128]
                        )
                        nc.vector.tensor_scalar_add(
                            xc0[:, p * 128 : (p + 1) * 128], H[:].bitcast(F32),
                            dep_c0[:, d0 + p : d0 + p + 1],
                        )

                    t0 = tc0_p.tile([128, G * 128], F32)
                    nc.scalar.activation(t0[:], xc0[:], AF.Tanh, scale=1.0 / C)
                    t1 = tc1_p.tile([128, G * S], F32)
                    nc.scalar.activation(t1[:], x_ps[:], AF.Tanh, scale=1.0 / C)
                    e0 = ec0_p.tile([128, G * 128], F16)
                    nc.scalar.activation(e0[:], t0[:], AF.Exp, bias=logm0[:], scale=C)
                    e1 = ec1_p.tile([128, G * S], F16)
                    nc.scalar.activation(e1[:], t1[:], AF.Exp, bias=logm1[:], scale=C)
                    nc.vector.tensor_tensor(out=e0[:], in0=e0[:], in1=tri0[:], op=OP.mult)
                    nc.vector.tensor_tensor(out=e1[:], in0=e1[:], in1=tri1[:], op=OP.mult)

                    red = red_p.tile([128, 4 * G], F32)  # i0 cols [0,2G), i1 [2G,4G)
                    for p in range(G):
                        dl = d0 + p
                        rcols0 = il[0][:, 2 * dl : 2 * dl + 2]
                        rcols1 = il[1][:, 2 * dl : 2 * dl + 2]
                        # i-chunk 1 (i in [128,256)): only j-chunk1 contributes
                        nc.tensor.matmul(
                            red[:, 2 * G + 2 * p : 2 * G + 2 * p + 2],
                            e1[:, p * S + 128 : p * S + S], rcols1,
                            start=True, stop=True,
                        )
                        # i-chunk 0: j-chunk0 + j-chunk1
                        nc.tensor.matmul(
                            red[:, 2 * p : 2 * p + 2],
                            e0[:, p * 128 : (p + 1) * 128], rcols0,
                            start=True, stop=False,
                        )
                        nc.tensor.matmul(
                            red[:, 2 * p : 2 * p + 2],
                            e1[:, p * S : p * S + 128], rcols1,
                            start=False, stop=True,
                        )
                    nc.vector.tensor_copy(
                        sumsW[0][:, 2 * d0 : 2 * d0 + 2 * G], red[:, 0 : 2 * G]
                    )
                    nc.vector.tensor_copy(
                        sumsW[1][:, 2 * d0 : 2 * d0 + 2 * G], red[:, 2 * G : 4 * G]
                    )

            # ---------- phase C ----------
            with (
                tc.tile_pool(name="pc_sb", bufs=2) as pc_sb,
                tc.tile_pool(name="pc_ps", bufs=2, space="PSUM") as pc_ps,
                tc.tile_pool(name="pc_keep", bufs=1) as pc_keep,
                tc.tile_pool(name="dram", bufs=1, space="DRAM") as dram,
            ):
                # attn natural per i-chunk: W/(sums + (sums==0))
                attn_nat = [
                    pc_keep.tile([128, HALF], F32, tag=f"an{i}", name=f"an{i}") for i in range(2)
                ]
                for i in range(2):
                    v3 = sumsW[i][:].rearrange("p (d two) -> p d two", two=2)
                    sums_v = v3[:, :, 0:1]
                    w_v = v3[:, :, 1:2]
                    s2 = pc_sb.tile([128, HALF], F32, tag="s2", name="s2")
                    nc.vector.scalar_tensor_tensor(
                        out=s2[:].unsqueeze(2), in0=sums_v, scalar=0.0,
                        in1=sums_v, op0=OP.is_equal, op1=OP.add,
                    )
                    rcp = pc_sb.tile([128, HALF], F32, tag="rcp", name="rcp")
                    nc.vector.reciprocal(out=rcp[:], in_=s2[:])
                    nc.vector.tensor_tensor(
                        out=attn_nat[i][:].unsqueeze(2), in0=w_v,
                        in1=rcp[:].unsqueeze(2), op=OP.mult,
                    )

                # transpose attn_nat -> attnT_half [150, 256]
                attnT_h = [
                    pc_keep.tile([n, S], F32R, tag=f"ath{i}", name=f"ath{i}")
                    for i, (o, n) in enumerate(DM)
                ]
                for i, (o, n) in enumerate(DM):
                    for ic in range(2):
                        tp = pc_ps.tile([n, 128], F32, tag="tp", name="tp")
                        nc.tensor.transpose(
                            tp[:], attn_nat[ic][:, o : o + n], ident[:]
                        )
                        nc.vector.tensor_copy(
                            attnT_h[i][:, ic * 128 : (ic + 1) * 128], tp[:]
                        )

                # pairwise AllGather: [150, 256] -> [300, 256]
                ag_in = dram.tile([HALF, S], F32R)
                ag_out = dram.tile([D, S], F32R)
                for i, (o, n) in enumerate(DM):
                    nc.sync.dma_start(ag_in[o : o + n, :], attnT_h[i][:])
                nc.gpsimd.collective_compute(
                    "AllGather",
                    mybir.AluOpType.bypass,
                    replica_groups=[[0, 1], [2, 3], [4, 5], [6, 7]],
                    ins=[ag_in.opt()],
                    outs=[ag_out.opt()],
                )
                attnT = [
                    pc_keep.tile([n, S], F32R, tag=f"atf{i}", name=f"atf{i}")
                    for i, (o, n) in enumerate(DC)
                ]
                for i, (o, n) in enumerate(DC):
                    nc.sync.dma_start(attnT[i][:], ag_out[o : o + n, :])

                # mask row broadcast (0.5*rep_mask over s)
                Mb = pc_keep.tile([128, S], F32)
                nc.gpsimd.partition_broadcast(Mb[:], mh_row[0:1, :])

                # gate^T + blend per g-chunk
                for i, (o, n) in enumerate(DC):
                    ps = pc_ps.tile([n, S], F32, tag="gps", name="gps")
                    for k in range(3):
                        nc.tensor.matmul(
                            ps[:], Wf1T[k][:, o : o + n],
                            repT[k][:],
                            start=(k == 0), stop=False,
                        )
                        nc.tensor.matmul(
                            ps[:], Wf2T[k][:, o : o + n],
                            attnT[k][:],
                            start=False, stop=False,
                        )
                    nc.tensor.matmul(
                        ps[:], b_f_row[0:1, o : o + n],
                        ones_row[0:1, 0:S],
                        start=False, stop=True,
                    )
                    th = pc_sb.tile([n, S], F32, tag="th", name="th")
                    nc.scalar.activation(th[:], ps[:], AF.Tanh, scale=0.5)

                    diff = pc_sb.tile([n, S], F32, tag="diff", name="diff")
                    nc.vector.tensor_tensor(
                        out=diff[:], in0=repT[i][:].bitcast(F32), in1=attnT[i][:].bitcast(F32), op=OP.subtract
                    )
                    summ = pc_sb.tile([n, S], F32, tag="summ", name="summ")
                    nc.vector.tensor_tensor(
                        out=summ[:], in0=repT[i][:].bitcast(F32), in1=attnT[i][:].bitcast(F32), op=OP.add
                    )
                    nc.vector.tensor_tensor(
                        out=diff[:], in0=th[:], in1=diff[:], op=OP.mult
                    )
                    nc.vector.tensor_tensor(
                        out=summ[:], in0=summ[:], in1=diff[:], op=OP.add
                    )
                    nc.vector.tensor_tensor(
                        out=summ[:], in0=summ[:], in1=Mb[0:n, :], op=OP.mult
                    )
                    nc.sync.dma_start(outT_d[o : o + n, :], summ[:])

    nc.compile()
    return nc


def _host_prep(inputs, rep_mask, W_fc, b_fc, W1, W2, b1, W_f1, W_f2, b_f):
    f = np.float32
    tri0 = (np.arange(128)[:, None] > np.arange(128)[None, :]).astype(np.float16)
    tri1 = ((np.arange(128, 256))[:, None] > np.arange(S)[None, :]).astype(np.float16)
    tri0g = np.tile(tri0, (1, G))
    tri1g = np.tile(tri1, (1, G))
    in_maps = []
    for c in range(8):
        b, h = c // 2, c % 2
        lo = h * HALF
        rm = rep_mask[b].astype(f)
        in_maps.append({
            "inputsT": np.ascontiguousarray(inputs[b].T, dtype=f),
            "W_fcT": np.ascontiguousarray(W_fc.T, dtype=f),
            "W_fcTh": np.ascontiguousarray(W_fc.T[:, lo : lo + HALF], dtype=f),
            "b_fch_row": b_fc[lo : lo + HALF].reshape(1, HALF).astype(f),
            "ident": np.eye(128, dtype=f),
            "ones_row": np.ones((1, D), dtype=f),
            "W1Th": np.ascontiguousarray(W1.T[:, lo : lo + HALF], dtype=f),
            "W2Th": np.ascontiguousarray(W2.T[:, lo : lo + HALF], dtype=f),
            "W_f1T": np.ascontiguousarray(W_f1.T, dtype=f),
            "W_f2T": np.ascontiguousarray(W_f2.T, dtype=f),
            "b_fc_row": b_fc.reshape(1, D).astype(f),
            "b1h_row": b1[lo : lo + HALF].reshape(1, HALF).astype(f),
            "b_f_row": b_f.reshape(1, D).astype(f),
            "logm": ((rm - 1.0) * (-NEG)).reshape(S, 1).astype(f),
            "mh_row": (0.5 * rm).reshape(1, S).astype(f),
            "tri0": tri0g,
            "tri1": tri1g,
        })
    return in_maps


def kernel(**inputs):
    from concourse.bass_utils import run_bass_kernel_spmd

    if "nc" not in _CACHE:
        _CACHE["nc"] = _build_nc()
    nc = _CACHE["nc"]

    in_maps = _host_prep(**inputs)
    res = run_bass_kernel_spmd(nc, in_maps, list(range(8)))
    out = np.stack(
        [res.results[2 * b]["outT"].T for b in range(B)], axis=0
    ).astype(np.float32)
    return out


# revision 15
# speedup vs baseline: 1.0415x; 1.0415x over previous
"""DiSA (directional self-attention) Bass kernel for Trainium2, 8 cores.

Math (per batch b):
  rep = elu(inputs @ W_fc.T + b_fc)                       [S, D]
  dep = rep @ W1.T ; head = rep @ W2.T                    [S, D]
  logits[i,j,d] = C*tanh((dep[j,d] + head[i,d] + b1[d])/C)
  mask[i,j] = rep_mask[j] * (j > i)
  attn = masked softmax over j, per (i, d) channel  (shift-invariance:
         logits bounded in [-C, C], so no max-subtract needed)
  attn_res[i,d] = sum_j attn * rep[j,d]
  gate = sigmoid(rep @ W_f1.T + attn_res @ W_f2.T + b_f)
       = 0.5 + 0.5*tanh(0.5*z)
  out = (gate*rep + (1-gate)*attn_res) * rep_mask[i]
      = 0.5*rep_mask[i] * ((rep+attn_res) + tanh(0.5*z)*(rep-attn_res))

Sharding: core c -> batch b=c//2, d-half h=c%2 (planes d in [150h, 150h+150)).
Each core computes the full out[b].T (phase C duplicated in the pair after a
pairwise AllGather of attn_res.T); the host takes core 2b's output.

Per-d-plane layout: [j (partitions), i (free)].  exp(masked logits) is built
as exp(C*tanh(x/C) + logmask[j]) (rep_mask folded into the ACT bias); the
strict-upper triangle mask (j > i) is a constant fp16 multiply; both
softmax reductions over j (sum of e, sum of e*rep) are PE matmuls with the
masked-exp tile as the stationary operand and [ones | rep_col] as a 2-column
moving operand, so results land as [i, 2] PSUM columns.

All matmul operands are fp16 (PE 1 cycle/row; fp32 PSUM accumulation); the
tanh input x = dep16 + head16 is summed in fp32 PSUM so only the fp16
rounding of dep/head (~1.5e-3 abs) enters the exponent.
"""

import numpy as np

B, S, D = 4, 256, 300
C = 5.0
HALF = D // 2          # 150 d-planes per core
G = 6                  # planes per group
NG = HALF // G         # 25 groups
NEG = -30000.0         # exp(x + NEG) == 0 in fp32

_CACHE: dict = {}


def _chunks(total, step=128):
    return [(s, min(step, total - s)) for s in range(0, total, step)]


def _build_nc():
    import concourse.bass as bass
    import concourse.tile as tile
    from concourse import bacc, mybir

    F32 = mybir.dt.float32
    F16 = mybir.dt.float16
    AF = mybir.ActivationFunctionType
    OP = mybir.AluOpType

    nc = bacc.Bacc("TRN2", target_bir_lowering=False, debug=False, num_devices=8)

    def din(name, shape, dt=F16):
        return nc.dram_tensor(name, shape, dt, kind="ExternalInput").ap()

    inputsT_d = din("inputsT", [D, S])          # inputs[b].T
    W_fcT_d = din("W_fcT", [D, D])              # [e, h]
    W_fcTh_d = din("W_fcTh", [D, HALF])         # W_fc.T[:, half]
    b_fch_d = din("b_fch_row", [1, HALF])
    ones_d = din("ones_row", [1, D])
    ident_d = din("ident", [128, 128])
    W1T_d = din("W1Th", [D, HALF])              # W1.T[:, half]
    W2T_d = din("W2Th", [D, HALF])
    W_f1T_d = din("W_f1T", [D, D])
    W_f2T_d = din("W_f2T", [D, D])
    b_fc_d = din("b_fc_row", [1, D])
    b1h_d = din("b1h_row", [1, HALF])
    b_f_d = din("b_f_row", [1, D])
    logm_d = din("logm", [S, 1], F32)           # (rep_mask-1)*30000
    mh_d = din("mh_row", [1, S], F32)           # 0.5*rep_mask
    tri0_d = din("tri0", [128, G * 128])        # (j>i) tile, j in [0,128)
    tri1_d = din("tri1", [128, G * S])          # (j>i), j in [128,256)
    outT_d = nc.dram_tensor("outT", [D, S], F32, kind="ExternalOutput").ap()

    DC = _chunks(D)          # [(0,128),(128,128),(256,44)]
    DM = _chunks(HALF)       # [(0,128),(128,22)]

    with tile.TileContext(nc) as tc:
        # ---------- persistent tiles ----------
        with (
            tc.tile_pool(name="persist", bufs=1) as pp,
            tc.tile_pool(name="sumsw", bufs=1) as swp,
        ):
            ones_row = pp.tile([1, D], F16)
            nc.sync.dma_start(ones_row[:], ones_d[:])

            inT = [pp.tile([n, S], F16, tag=f"inT{i}", name=f"inT{i}") for i, (o, n) in enumerate(DC)]
            WfcT = [pp.tile([n, D], F16, tag=f"wfc{i}", name=f"wfc{i}") for i, (o, n) in enumerate(DC)]
            WfcTh = [pp.tile([n, HALF], F16, tag=f"wfch{i}", name=f"wfch{i}") for i, (o, n) in enumerate(DC)]
            W1T = [pp.tile([n, HALF], F16, tag=f"w1{i}", name=f"w1_{i}") for i, (o, n) in enumerate(DC)]
            W2T = [pp.tile([n, HALF], F16, tag=f"w2{i}", name=f"w2_{i}") for i, (o, n) in enumerate(DC)]
            Wf1T = [pp.tile([n, D], F16, tag=f"wg1{i}", name=f"wg1_{i}") for i, (o, n) in enumerate(DC)]
            Wf2T = [pp.tile([n, D], F16, tag=f"wg2{i}", name=f"wg2_{i}") for i, (o, n) in enumerate(DC)]
            for i, (o, n) in enumerate(DC):
                nc.sync.dma_start(inT[i][:], inputsT_d[o : o + n, :])
                nc.sync.dma_start(WfcT[i][:], W_fcT_d[o : o + n, :])
                nc.scalar.dma_start(WfcTh[i][:], W_fcTh_d[o : o + n, :])
                nc.scalar.dma_start(W1T[i][:], W1T_d[o : o + n, :])
                nc.gpsimd.dma_start(W2T[i][:], W2T_d[o : o + n, :])
                nc.gpsimd.dma_start(Wf1T[i][:], W_f1T_d[o : o + n, :])
                nc.gpsimd.dma_start(Wf2T[i][:], W_f2T_d[o : o + n, :])
            b_fc_row = pp.tile([1, D], F16)
            nc.sync.dma_start(b_fc_row[:], b_fc_d[:])
            b_fch_row = pp.tile([1, HALF], F16)
            nc.sync.dma_start(b_fch_row[:], b_fch_d[:])
            ident = pp.tile([128, 128], F16)
            nc.sync.dma_start(ident[:], ident_d[:])
            b1h_row = pp.tile([1, HALF], F16)
            nc.sync.dma_start(b1h_row[:], b1h_d[:])
            b_f_row = pp.tile([1, D], F16)
            nc.sync.dma_start(b_f_row[:], b_f_d[:])
            logm0 = pp.tile([128, 1], F32)
            nc.sync.dma_start(logm0[:], logm_d[0:128, :])
            logm1 = pp.tile([128, 1], F32)
            nc.sync.dma_start(logm1[:], logm_d[128:256, :])
            mh_row = pp.tile([1, S], F32)
            nc.sync.dma_start(mh_row[:], mh_d[:])
            tri0 = pp.tile([128, G * 128], F16)
            nc.scalar.dma_start(tri0[:], tri0_d[:])
            tri1 = pp.tile([128, G * S], F16)
            nc.gpsimd.dma_start(tri1[:], tri1_d[:])

            # phase A outputs (persist through B/C)
            repT = [pp.tile([n, S], F16, tag=f"repT{i}", name=f"repT{i}") for i, (o, n) in enumerate(DC)]
            rep_nat = [pp.tile([128, HALF], F32, tag=f"repn{i}", name=f"repn{i}") for i in range(2)]
            depT = [pp.tile([n, S], F16, tag=f"depT{i}", name=f"depT{i}") for i, (o, n) in enumerate(DM)]
            headT = [pp.tile([n, S], F16, tag=f"headT{i}", name=f"headT{i}") for i, (o, n) in enumerate(DM)]
            dep_c0 = pp.tile([128, HALF], F32)     # dep natural, j in [0,128)
            il = [pp.tile([128, 2 * HALF], F16, tag=f"il{i}", name=f"il{i}") for i in range(2)]
            # phase B accumulators: cols (d_local, {sums, W})
            sumsW = [swp.tile([128, 2 * HALF], F32, tag=f"sw{i}", name=f"sw{i}") for i in range(2)]

            # ---------- phase A ----------
            with (
                tc.tile_pool(name="pa_ps", bufs=2, space="PSUM") as pa_ps,
                tc.tile_pool(name="pa_sb", bufs=2) as pa_sb,
            ):
                def elu_from_psum(ps_ap, out_ap, n):
                    # out = relu(x) + exp(min(x, 0)) - 1   (b_fc added in PSUM)
                    relu_t = pa_sb.tile([n, ps_ap.shape[1]], F32, tag="elu_r", name="elu_r")
                    nc.scalar.activation(relu_t[:], ps_ap, AF.Relu)
                    min_t = pa_sb.tile([n, ps_ap.shape[1]], F32, tag="elu_m", name="elu_m")
                    nc.vector.tensor_scalar(
                        out=min_t[:], in0=ps_ap, scalar1=0.0, scalar2=None, op0=OP.min
                    )
                    exp_t = pa_sb.tile([n, ps_ap.shape[1]], F32, tag="elu_e", name="elu_e")
                    nc.scalar.activation(exp_t[:], min_t[:], AF.Exp)
                    nc.vector.scalar_tensor_tensor(
                        out=out_ap, in0=exp_t[:], scalar=-1.0, in1=relu_t[:],
                        op0=OP.add, op1=OP.add,
                    )

                # rep^T [d, s] = elu(W_fcT.T @ inputsT + b_fc)
                for i, (o, n) in enumerate(DC):
                    ps = pa_ps.tile([n, S], F32, tag="paT", name="paT")
                    for k, (eo, en) in enumerate(DC):
                        nc.tensor.matmul(
                            ps[:], WfcT[k][:, o : o + n], inT[k][:],
                            start=(k == 0), stop=False,
                        )
                    nc.tensor.matmul(
                        ps[:], b_fc_row[0:1, o : o + n], ones_row[0:1, 0:S],
                        start=False, stop=True,
                    )
                    elu_from_psum(ps[:], repT[i][:], n)

                # rep natural half [s-chunk, d_local] = elu(inputsT.T @ W_fcTh + b_fch)
                for i in range(2):
                    so = 128 * i
                    ps = pa_ps.tile([128, HALF], F32, tag="paN", name="paN")
                    for k, (eo, en) in enumerate(DC):
                        nc.tensor.matmul(
                            ps[:], inT[k][:, so : so + 128], WfcTh[k][:],
                            start=(k == 0), stop=False,
                        )
                    nc.tensor.matmul(
                        ps[:], ones_row[0:1, 0:128], b_fch_row[:],
                        start=False, stop=True,
                    )
                    elu_from_psum(ps[:], rep_nat[i][:], 128)

                # interleave [ones | rep] fp16, per j-chunk
                for i in range(2):
                    v3 = il[i][:].rearrange("p (d two) -> p d two", two=2)
                    nc.vector.memset(v3[:, :, 0:1], 1.0)
                    nc.vector.tensor_copy(
                        v3[:, :, 1:2],
                        rep_nat[i][:].unsqueeze(2),
                    )

                # dep^T / head^T [d_local, s]
                for i, (o, n) in enumerate(DM):
                    ps = pa_ps.tile([n, S], F32, tag="paT", name="paT")
                    for k, (ho, hn) in enumerate(DC):
                        nc.tensor.matmul(
                            ps[:], W1T[k][:, o : o + n], repT[k][:],
                            start=(k == 0), stop=(k == 2),
                        )
                    nc.vector.tensor_copy(depT[i][:], ps[:])

                    ps2 = pa_ps.tile([n, S], F32, tag="paT", name="paT")
                    for k, (ho, hn) in enumerate(DC):
                        nc.tensor.matmul(
                            ps2[:], W2T[k][:, o : o + n], repT[k][:],
                            start=(k == 0), stop=False,
                        )
                    nc.tensor.matmul(
                        ps2[:], b1h_row[0:1, o : o + n], ones_row[0:1, 0:S],
                        start=False, stop=True,
                    )
                    nc.vector.tensor_copy(headT[i][:], ps2[:])

                # dep natural c0 [j in 0:128, d_local]
                ps = pa_ps.tile([128, HALF], F32, tag="paN", name="paN")
                for k, (ho, hn) in enumerate(DC):
                    nc.tensor.matmul(
                        ps[:], repT[k][:, 0:128], W1T[k][:],
                        start=(k == 0), stop=(k == 2),
                    )
                nc.vector.tensor_copy(dep_c0[:], ps[:])

            # ---------- phase B ----------
            def rows_of(tiles, lo, hi):
                """Split [lo,hi) d_local rows across the DM tiles."""
                segs = []
                for i, (o, n) in enumerate(DM):
                    a, b2 = max(lo, o), min(hi, o + n)
                    if a < b2:
                        segs.append((tiles[i], a - o, b2 - a))
                return segs

            with (
                tc.tile_pool(name="stA", bufs=3) as stA_p,
                tc.tile_pool(name="stB", bufs=3) as stB_p,
                tc.tile_pool(name="Hb", bufs=3) as H_p,
                tc.tile_pool(name="xc0", bufs=2) as xc0_p,
                tc.tile_pool(name="xps", bufs=2, space="PSUM") as xps_p,
                tc.tile_pool(name="redps", bufs=2, space="PSUM") as red_p,
                tc.tile_pool(name="tc0", bufs=2) as tc0_p,
                tc.tile_pool(name="tc1", bufs=2) as tc1_p,
                tc.tile_pool(name="ec0", bufs=2) as ec0_p,
                tc.tile_pool(name="ec1", bufs=2) as ec1_p,
            ):
                for grp in range(NG):
                    d0 = grp * G
                    stageA = stA_p.tile([1, G * S], F16)
                    off = 0
                    for t, ro, rn in rows_of(headT, d0, d0 + G):
                        nc.sync.dma_start(
                            stageA[0:1, off : off + rn * S], t[ro : ro + rn, :]
                        )
                        off += rn * S
                    stageB = stB_p.tile([1, G * 128], F16)
                    off = 0
                    for t, ro, rn in rows_of(depT, d0, d0 + G):
                        nc.sync.dma_start(
                            stageB[0:1, off : off + rn * 128], t[ro : ro + rn, 128:S]
                        )
                        off += rn * 128

                    x_ps = xps_p.tile([128, G * S], F32)
                    xc0 = xc0_p.tile([128, G * 128], F32)
                    for p in range(G):
                        o1 = p * S
                        nc.tensor.matmul(
                            x_ps[:, o1 : o1 + S],
                            ones_row[0:1, 0:128],
                            stageA[0:1, o1 : o1 + S],
                            start=True, stop=False,
                        )
                        nc.tensor.matmul(
                            x_ps[:, o1 : o1 + S],
                            stageB[0:1, p * 128 : (p + 1) * 128],
                            ones_row[0:1, 0:S],
                            start=False, stop=True,
                        )
                        H = H_p.tile([128, 128], F16)
                        nc.gpsimd.partition_broadcast(
                            H[:], stageA[0:1, o1 : o1 + 128]
                        )
                        nc.vector.tensor_scalar_add(
                            xc0[:, p * 128 : (p + 1) * 128], H[:],
                            dep_c0[:, d0 + p : d0 + p + 1],
                        )

                    t0 = tc0_p.tile([128, G * 128], F32)
                    nc.scalar.activation(t0[:], xc0[:], AF.Tanh, scale=1.0 / C)
                    t1 = tc1_p.tile([128, G * S], F32)
                    nc.scalar.activation(t1[:], x_ps[:], AF.Tanh, scale=1.0 / C)
                    e0 = ec0_p.tile([128, G * 128], F16)
                    nc.scalar.activation(e0[:], t0[:], AF.Exp, bias=logm0[:], scale=C)
                    e1 = ec1_p.tile([128, G * S], F16)
                    nc.scalar.activation(e1[:], t1[:], AF.Exp, bias=logm1[:], scale=C)
                    nc.vector.tensor_tensor(out=e0[:], in0=e0[:], in1=tri0[:], op=OP.mult)
                    nc.vector.tensor_tensor(out=e1[:], in0=e1[:], in1=tri1[:], op=OP.mult)

                    red = red_p.tile([128, 4 * G], F32)  # i0 cols [0,2G), i1 [2G,4G)
                    for p in range(G):
                        dl = d0 + p
                        rcols0 = il[0][:, 2 * dl : 2 * dl + 2]
                        rcols1 = il[1][:, 2 * dl : 2 * dl + 2]
                        # i-chunk 1 (i in [128,256)): only j-chunk1 contributes
                        nc.tensor.matmul(
                            red[:, 2 * G + 2 * p : 2 * G + 2 * p + 2],
                            e1[:, p * S + 128 : p * S + S], rcols1,
                            start=True, stop=True,
                        )
                        # i-chunk 0: j-chunk0 + j-chunk1
                        nc.tensor.matmul(
                            red[:, 2 * p : 2 * p + 2],
                            e0[:, p * 128 : (p + 1) * 128], rcols0,
                            start=True, stop=False,
                        )
                        nc.tensor.matmul(
                            red[:, 2 * p : 2 * p + 2],
                            e1[:, p * S : p * S + 128], rcols1,
                            start=False, stop=True,
                        )
                    nc.vector.tensor_copy(
                        sumsW[0][:, 2 * d0 : 2 * d0 + 2 * G], red[:, 0 : 2 * G]
                    )
                    nc.vector.tensor_copy(
                        sumsW[1][:, 2 * d0 : 2 * d0 + 2 * G], red[:, 2 * G : 4 * G]
                    )

            # ---------- phase C ----------
            with (
                tc.tile_pool(name="pc_sb", bufs=2) as pc_sb,
                tc.tile_pool(name="pc_ps", bufs=2, space="PSUM") as pc_ps,
                tc.tile_pool(name="pc_gps", bufs=1, space="PSUM") as pc_gps,
                tc.tile_pool(name="pc_keep", bufs=1) as pc_keep,
                tc.tile_pool(name="dram", bufs=1, space="DRAM") as dram,
            ):
                # attn natural per i-chunk: W/(sums + (sums==0))
                attn_nat = [
                    pc_keep.tile([128, HALF], F16, tag=f"an{i}", name=f"an{i}") for i in range(2)
                ]
                for i in range(2):
                    v3 = sumsW[i][:].rearrange("p (d two) -> p d two", two=2)
                    sums_v = v3[:, :, 0:1]
                    w_v = v3[:, :, 1:2]
                    s2 = pc_sb.tile([128, HALF], F32, tag="s2", name="s2")
                    nc.vector.scalar_tensor_tensor(
                        out=s2[:].unsqueeze(2), in0=sums_v, scalar=0.0,
                        in1=sums_v, op0=OP.is_equal, op1=OP.add,
                    )
                    rcp = pc_sb.tile([128, HALF], F32, tag="rcp", name="rcp")
                    nc.vector.reciprocal(out=rcp[:], in_=s2[:])
                    nc.vector.tensor_tensor(
                        out=attn_nat[i][:].unsqueeze(2), in0=w_v,
                        in1=rcp[:].unsqueeze(2), op=OP.mult,
                    )

                # transpose attn_nat -> attnT_half [150, 256] (fp16)
                attnT_h = [
                    pc_keep.tile([n, S], F16, tag=f"ath{i}", name=f"ath{i}")
                    for i, (o, n) in enumerate(DM)
                ]
                for i, (o, n) in enumerate(DM):
                    for ic in range(2):
                        tp = pc_ps.tile([n, 128], F16, tag="tp", name="tp")
                        nc.tensor.transpose(
                            tp[:], attn_nat[ic][:, o : o + n], ident[:]
                        )
                        nc.vector.tensor_copy(
                            attnT_h[i][:, ic * 128 : (ic + 1) * 128], tp[:]
                        )

                # pairwise AllGather (fp16): [150, 256] -> [300, 256]
                ag_in = dram.tile([HALF, S], F16)
                ag_out = dram.tile([D, S], F16)
                for i, (o, n) in enumerate(DM):
                    nc.sync.dma_start(ag_in[o : o + n, :], attnT_h[i][:])
                # gate^T rep-part first: overlaps with the collective
                gps = [
                    pc_gps.tile([n, S], F32, tag=f"gps{i}", name=f"gps{i}")
                    for i, (o, n) in enumerate(DC)
                ]
                for i, (o, n) in enumerate(DC):
                    for k in range(3):
                        nc.tensor.matmul(
                            gps[i][:], Wf1T[k][:, o : o + n], repT[k][:],
                            start=(k == 0), stop=False,
                        )
                    nc.tensor.matmul(
                        gps[i][:], b_f_row[0:1, o : o + n], ones_row[0:1, 0:S],
                        start=False, stop=False,
                    )
                nc.gpsimd.collective_compute(
                    "AllGather",
                    mybir.AluOpType.bypass,
                    replica_groups=[[0, 1], [2, 3], [4, 5], [6, 7]],
                    ins=[ag_in.opt()],
                    outs=[ag_out.opt()],
                )
                attnT = [
                    pc_keep.tile([n, S], F16, tag=f"atf{i}", name=f"atf{i}")
                    for i, (o, n) in enumerate(DC)
                ]
                for i, (o, n) in enumerate(DC):
                    nc.sync.dma_start(attnT[i][:], ag_out[o : o + n, :])

                # mask row broadcast (0.5*rep_mask over s)
                Mb = pc_keep.tile([128, S], F32)
                nc.gpsimd.partition_broadcast(Mb[:], mh_row[0:1, :])

                # gate^T attn-part + tanh + blend per g-chunk
                for i, (o, n) in enumerate(DC):
                    for k in range(3):
                        nc.tensor.matmul(
                            gps[i][:], Wf2T[k][:, o : o + n], attnT[k][:],
                            start=False, stop=(k == 2),
                        )
                    th = pc_sb.tile([n, S], F32, tag="th", name="th")
                    nc.scalar.activation(th[:], gps[i][:], AF.Tanh, scale=0.5)

                    diff = pc_sb.tile([n, S], F32, tag="diff", name="diff")
                    nc.vector.tensor_tensor(
                        out=diff[:], in0=repT[i][:], in1=attnT[i][:], op=OP.subtract
                    )
                    summ = pc_sb.tile([n, S], F32, tag="summ", name="summ")
                    nc.vector.tensor_tensor(
                        out=summ[:], in0=repT[i][:], in1=attnT[i][:], op=OP.add
                    )
                    nc.vector.tensor_tensor(
                        out=diff[:], in0=th[:], in1=diff[:], op=OP.mult
                    )
                    nc.vector.tensor_tensor(
                        out=summ[:], in0=summ[:], in1=diff[:], op=OP.add
                    )
                    nc.vector.tensor_tensor(
                        out=summ[:], in0=summ[:], in1=Mb[0:n, :], op=OP.mult
                    )
                    nc.sync.dma_start(outT_d[o : o + n, :], summ[:])

    nc.compile()
    return nc


def _host_prep(inputs, rep_mask, W_fc, b_fc, W1, W2, b1, W_f1, W_f2, b_f):
    f = np.float32
    h = np.float16
    tri0 = (np.arange(128)[:, None] > np.arange(128)[None, :]).astype(h)
    tri1 = ((np.arange(128, 256))[:, None] > np.arange(S)[None, :]).astype(h)
    tri0g = np.tile(tri0, (1, G))
    tri1g = np.tile(tri1, (1, G))
    in_maps = []
    for c in range(8):
        b, hh = c // 2, c % 2
        lo = hh * HALF
        rm = rep_mask[b].astype(f)
        in_maps.append({
            "inputsT": np.ascontiguousarray(inputs[b].T).astype(h),
            "W_fcT": np.ascontiguousarray(W_fc.T).astype(h),
            "W_fcTh": np.ascontiguousarray(W_fc.T[:, lo : lo + HALF]).astype(h),
            "b_fch_row": b_fc[lo : lo + HALF].reshape(1, HALF).astype(h),
            "ident": np.eye(128, dtype=h),
            "ones_row": np.ones((1, D), dtype=h),
            "W1Th": np.ascontiguousarray(W1.T[:, lo : lo + HALF]).astype(h),
            "W2Th": np.ascontiguousarray(W2.T[:, lo : lo + HALF]).astype(h),
            "W_f1T": np.ascontiguousarray(W_f1.T).astype(h),
            "W_f2T": np.ascontiguousarray(W_f2.T).astype(h),
            "b_fc_row": b_fc.reshape(1, D).astype(h),
            "b1h_row": b1[lo : lo + HALF].reshape(1, HALF).astype(h),
            "b_f_row": b_f.reshape(1, D).astype(h),
            "logm": ((rm - 1.0) * (-NEG)).reshape(S, 1).astype(f),
            "mh_row": (0.5 * rm).reshape(1, S).astype(f),
            "tri0": tri0g,
            "tri1": tri1g,
        })
    return in_maps


def kernel(**inputs):
    from concourse.bass_utils import run_bass_kernel_spmd

    if "nc" not in _CACHE:
        _CACHE["nc"] = _build_nc()
    nc = _CACHE["nc"]

    in_maps = _host_prep(**inputs)
    res = run_bass_kernel_spmd(nc, in_maps, list(range(8)))
    out = np.stack(
        [res.results[2 * b]["outT"].T for b in range(B)], axis=0
    ).astype(np.float32)
    return out
